# revision 38
# baseline (speedup 1.0000x reference)
"""3-layer GCN (EnhancedGraphNeuralNetwork) on 8 Trainium2 NeuronCores.

Strategy (dst-node sharded, graph-parallel per the sharding hint):
  - Host: add self loops, compute in-degrees, relabel nodes by descending
    degree, split 128-node blocks round-robin across 8 cores, and pack
    each block's incoming edges into 128-edge chunks (bucketed by 32k-row
    source windows for int16 gather indices, sorted ascending within each
    segment for HBM locality). Chunk counts are shared across cores so
    all cores run one SPMD program.
  - Key algebraic fact: aggregation commutes with the layer matmul:
       segsum((dinv*x)[src]) @ W  ==  segsum(((dinv*x) @ W)[src])
    so each layer gathers raw (dinv-scaled) features from a bf16 node
    table via dma_gather (4 SWDGE queues round-robin), segment-sums
    chunks into PSUM with one-hot selection matmuls (S built batched on
    DVE per contiguous chunk run), applies dinv on the dst side,
    transposes, and runs one [128,128]x[128,H] matmul per block.
  - The layer-1 gather table (dinv * x, bf16) is precomputed on the HOST
    and passed in replicated, killing the first AllGather entirely; the
    transposed residual is likewise a host input.
  - Tables use a window-major row layout (t = w*32768 + c*szw + r) so
    each 32k-row gather window is one AllGather piece.
  - Consumer-side BatchNorm: each block's RAW pre-BN Z is transposed and
    shipped to its window's AllGather piece as soon as the block's
    columns are computed, and the piece fires mid-layer the moment its
    32 blocks are shipped — so the 26MB of inter-layer communication
    overlaps the producing layer's own gather/matmul work instead of
    serializing after it.  BN stats stay hardware bn_stats + a 1KB
    AllReduce; the affine, residual (recovered as table0*sqrt(deg)),
    relu, and source-side dinv are applied by every core while building
    the full next-layer table locally from the AllGathered raw Z
    (per-feature coefficients partition-broadcast via a tiny DRAM
    roundtrip; relu fused after the dinv multiply since dinv > 0).
"""

import math
import numpy as np
import ml_dtypes

import concourse.bass as bass
import concourse.bacc as bacc
import concourse.tile as tile
import concourse.mybir as mybir
from concourse.bass_utils import run_bass_kernel_spmd

N_CORES = 8
P = 128
EPS = 1e-5
WIN = 32768          # int16-addressable source window (table rows)
GROUP = 4            # dst blocks per gather group

FP = mybir.dt.float32
BF = mybir.dt.bfloat16
I16 = mybir.dt.int16

PADLOC = 1000.0      # dstloc value for padding entries (kills one-hot row)


# ---------------------------------------------------------------- host prep

def _host_prep(x, edge_index, n_nodes):
    """Relabel, bucket edges by (core, block, window), pack gather plan."""
    N = n_nodes
    NPAD = ((N + (P * N_CORES) - 1) // (P * N_CORES)) * (P * N_CORES)
    J = NPAD // P // N_CORES          # blocks per core
    SH = J * P                        # nodes per core shard
    NW = (NPAD + WIN - 1) // WIN

    src = np.concatenate([edge_index[0], np.arange(N, dtype=np.int64)])
    dst = np.concatenate([edge_index[1], np.arange(N, dtype=np.int64)])

    deg = np.bincount(dst, minlength=N).astype(np.int64)  # >=1 (self loops)
    order = np.argsort(-deg, kind="stable")               # new id -> old id
    newid_of = np.empty(N, dtype=np.int64)
    newid_of[order] = np.arange(N)
    deg_new = np.ones(NPAD, dtype=np.int64)               # pad nodes: deg 1
    deg_new[:N] = deg[order]

    nsrc = newid_of[src]
    ndst = newid_of[dst]

    # table order: node n (new id) -> table row t(n).  Window-major layout:
    # shard row s of core c lands in gather-window w = s // WPC at
    # t = w*WIN + c*szw + (s - w*WPC), so each window is one AllGather piece.
    WPC = WIN // N_CORES                   # shard rows per full window
    g = np.arange(NPAD) // P
    s_all = (g // N_CORES) * P + (np.arange(NPAD) % P)   # shard row
    c_all = g % N_CORES
    w_all = np.minimum(s_all // WPC, (SH - 1) // WPC)
    szw = np.where(w_all < SH // WPC, WPC, SH - (SH // WPC) * WPC)
    t_all = w_all * WIN + c_all * szw + (s_all - w_all * WPC)

    # edge fields
    e_t = t_all[nsrc]                  # table row of source
    e_w = e_t // WIN                   # source window
    e_rel = (e_t % WIN).astype(np.int32)
    e_g = ndst // P                    # dst global block
    e_c = (e_g % N_CORES).astype(np.int32)
    e_j = (e_g // N_CORES).astype(np.int32)
    e_p = (ndst % P).astype(np.int32)  # dst local id

    # sort edges by (core, block, window), then ascending source row within
    # each segment (better HBM locality for the gather)
    key = ((e_c * J + e_j) * NW + e_w).astype(np.int64)
    o = np.lexsort((e_rel, key))
    ks, rels, ps = key[o], e_rel[o], e_p[o]
    bound = np.searchsorted(ks, np.arange(N_CORES * J * NW + 1))

    def seg(c, j, w):
        k = (c * J + j) * NW + w
        return bound[k], bound[k + 1]

    # group blocks
    groups = []
    j0 = 0
    while j0 < J:
        gs = min(GROUP, J - j0)
        groups.append(list(range(j0, j0 + gs)))
        j0 += gs

    # build plan + per-core packed arrays
    plan = []
    idx16 = [[] for _ in range(N_CORES)]   # per core: [128, m/16] int16 parts
    dloc = [[] for _ in range(N_CORES)]    # per core: [128, m/128] f32 parts
    gcol = 0
    for blocks in groups:
        calls = []
        blk_chunks = {j: [] for j in blocks}
        kstart = 0
        for w in range(NW):
            ns = {j: [seg(c, j, w) for c in range(N_CORES)] for j in blocks}
            m_j = {}
            for j in blocks:
                mx = max(b - a for a, b in ns[j])
                m_j[j] = ((mx + P - 1) // P) * P
            nidx = sum(m_j.values())
            if nidx == 0:
                continue
            # split into <=1024-index calls (SWDGE ring limit)
            o16 = sum(c_[2] for c_ in calls)  # int16 cols so far this group
            done = 0
            while done < nidx:
                piece = min(1024, nidx - done)
                calls.append((w, o16 + done // 16, piece // 16,
                              kstart + done // P))
                done += piece
            for c in range(N_CORES):
                vals = np.zeros(nidx, np.int32)
                dls = np.full(nidx, PADLOC, np.float32)
                off = 0
                for j in blocks:
                    a, b = ns[j][c]
                    n = b - a
                    vals[off:off + n] = rels[a:b]
                    dls[off:off + n] = ps[a:b]
                    off += m_j[j]
                wrapped = vals.reshape(nidx // 16, 16).T.astype(np.int16)
                idx16[c].append(np.tile(wrapped, (8, 1)))
                dloc[c].append(dls.reshape(nidx // P, P).T)
            boff = 0
            for j in blocks:
                nch = m_j[j] // P
                for i in range(nch):
                    blk_chunks[j].append(kstart + boff // P + i)
                boff += m_j[j]
            kstart += nidx // P
            gcol += nidx // P
        plan.append(dict(blocks=blocks, calls=calls, slots=kstart,
                         chunks=blk_chunks))

    idx16 = np.stack([np.concatenate(idx16[c], axis=1)
                      for c in range(N_CORES)])
    dloc = np.stack([np.concatenate(dloc[c], axis=1) for c in range(N_CORES)])

    # per-core shard data in table order
    xs = np.zeros((N_CORES, SH, x.shape[1]), dtype=np.float32)
    degt = np.ones((N_CORES, P, J), dtype=np.float32)
    for c in range(N_CORES):
        gbs = np.arange(J) * N_CORES + c
        nids = (gbs[:, None] * P + np.arange(P)[None, :]).reshape(-1)
        real = nids < N
        xr = np.zeros((SH, x.shape[1]), dtype=np.float32)
        xr[real] = x[order[nids[real]]]
        xs[c] = xr
        degt[c] = deg_new[nids].reshape(J, P).T.astype(np.float32)

    # host-precomputed layer-1 gather table (dinv * x, table row order) and
    # per-core transposed residual
    dinv_sh = 1.0 / np.sqrt(
        degt.transpose(0, 2, 1).reshape(N_CORES, SH))     # [C, SH]
    sg = np.arange(SH)
    wg = np.minimum(sg // WPC, (SH - 1) // WPC)
    szg = np.where(wg < SH // WPC, WPC, SH - (SH // WPC) * WPC)
    table0 = np.zeros((NPAD, x.shape[1]), dtype=ml_dtypes.bfloat16)
    for c in range(N_CORES):
        tg = wg * WIN + c * szg + (sg - wg * WPC)
        table0[tg] = (xs[c] * dinv_sh[c][:, None]).astype(ml_dtypes.bfloat16)
    xts = np.ascontiguousarray(xs.transpose(0, 2, 1))     # [C, F, SH] f32

    # degrees in table-row order, partition-major tiles: deg_pm[p, a] is
    # table row a*128+p (for consumer-side BN/relu/dinv table builds)
    degfull = np.ones(NPAD, np.float32)
    degfull[t_all] = deg_new.astype(np.float32)
    deg_pm = np.ascontiguousarray(degfull.reshape(NPAD // P, P).T)

    nfull = SH // WPC
    wsz = [WPC] * nfull + ([SH - nfull * WPC] if SH % WPC else [])
    meta = dict(N=N, NPAD=NPAD, J=J, SH=SH, NW=NW, plan=plan,
                o16_total=idx16.shape[2], slots_total=dloc.shape[2],
                order=order, wsz=wsz, deg_pm=deg_pm)
    return meta, idx16, dloc.astype(ml_dtypes.bfloat16), xs, degt, table0, xts


# ---------------------------------------------------------------- device

def _build(meta, hid, n_cls, stage=99, ldepth=99, warm=False, gbufs=2,
           zbf=False, psb=2, spb=4):
    """Build the SPMD bass program for all 8 cores."""
    J, SH = meta["J"], meta["SH"]
    N, NPAD = meta["N"], meta["NPAD"]
    plan = meta["plan"]
    F = hid
    O16, SLOTS = meta["o16_total"], meta["slots_total"]
    SLOTS_MAX = max(pl["slots"] for pl in plan)
    O16_MAX = max(sum(c[2] for c in pl["calls"]) for pl in plan)
    RUNMAX = 1
    for pl in plan:
        for j in pl["blocks"]:
            run = 1
            ch = pl["chunks"][j]
            for i in range(1, len(ch)):
                run = run + 1 if ch[i] == ch[i - 1] + 1 else 1
                RUNMAX = max(RUNMAX, run)

    nc = bacc.Bacc("TRN2", target_bir_lowering=False, debug=False,
                   num_devices=N_CORES, num_swdge_queues=4)

    table0_d = nc.dram_tensor("table0", [NPAD, F], BF, kind="ExternalInput")
    degpm_d = nc.dram_tensor("deg_pm", [P, NPAD // P], FP,
                             kind="ExternalInput")
    degt_d = nc.dram_tensor("degt", [P, J], FP, kind="ExternalInput")
    idx_d = nc.dram_tensor("idx16", [P, O16], I16, kind="ExternalInput")
    dloc_d = nc.dram_tensor("dloc", [P, SLOTS], BF, kind="ExternalInput")
    ident_d = nc.dram_tensor("ident", [P, P], FP, kind="ExternalInput")
    iota_d = nc.dram_tensor("iotar", [P, P], BF, kind="ExternalInput")
    W1_d = nc.dram_tensor("W1", [F, F], FP, kind="ExternalInput")
    W2_d = nc.dram_tensor("W2", [F, F], FP, kind="ExternalInput")
    W3_d = nc.dram_tensor("W3", [F, n_cls], FP, kind="ExternalInput")
    b1_d = nc.dram_tensor("b1", [F], FP, kind="ExternalInput")
    b2_d = nc.dram_tensor("b2", [F], FP, kind="ExternalInput")
    b3_d = nc.dram_tensor("b3", [n_cls], FP, kind="ExternalInput")
    g1_d = nc.dram_tensor("g1", [F], FP, kind="ExternalInput")
    be1_d = nc.dram_tensor("be1", [F], FP, kind="ExternalInput")
    g2_d = nc.dram_tensor("g2", [F], FP, kind="ExternalInput")
    be2_d = nc.dram_tensor("be2", [F], FP, kind="ExternalInput")
    out_d = nc.dram_tensor("out", [P, J, n_cls], FP, kind="ExternalOutput")

    with tile.TileContext(nc) as tc:
        with (
            tc.tile_pool(name="persist", bufs=1) as pp,
            tc.tile_pool(name="blk", bufs=3) as bp,
            tc.tile_pool(name="spool", bufs=spb) as sp,
            tc.tile_pool(name="gath", bufs=gbufs) as gp,
            tc.tile_pool(name="psum", bufs=psb, space="PSUM") as psp,
            tc.tile_pool(name="dram", bufs=1, space="DRAM") as dp,
        ):
            # ---------- constants
            ident = pp.tile([P, P], FP, tag="ident")
            nc.sync.dma_start(out=ident[:], in_=ident_d[:])
            ident_bf = pp.tile([P, P], BF, tag="identbf")
            nc.vector.tensor_copy(out=ident_bf[:], in_=ident[:])
            iotar = pp.tile([P, P], BF, tag="iotar")
            nc.sync.dma_start(out=iotar[:], in_=iota_d[:])

            w1 = pp.tile([F, F], BF, tag="w1")
            w2 = pp.tile([F, F], BF, tag="w2")
            w3 = pp.tile([F, n_cls], BF, tag="w3")
            nc.gpsimd.dma_start(out=w1[:], in_=W1_d[:])
            nc.gpsimd.dma_start(out=w2[:], in_=W2_d[:])
            nc.gpsimd.dma_start(out=w3[:], in_=W3_d[:])

            def col(dram1d, n=F):
                t = pp.tile([n, 1], FP, tag=f"col_{dram1d.name}")
                nc.sync.dma_start(out=t[:], in_=dram1d[:, None])
                return t

            b1c, b2c = col(b1_d), col(b2_d)
            g1c, be1c, g2c, be2c = col(g1_d), col(be1_d), col(g2_d), col(be2_d)
            b3bc = pp.tile([P, n_cls], FP, tag="b3bc")
            nc.gpsimd.dma_start(
                out=b3bc[:],
                in_=bass.AP(tensor=b3_d, offset=0, ap=[[0, P], [1, n_cls]]))
            epsc = pp.tile([P, 1], FP, tag="eps")
            nc.vector.memset(epsc[:], EPS)

            degt = pp.tile([P, J], FP, tag="degt")
            nc.sync.dma_start(out=degt[:], in_=degt_d[:])
            dinv = pp.tile([P, J], FP, tag="dinv")
            nc.scalar.activation(out=dinv[:], in_=degt[:],
                                 func=mybir.ActivationFunctionType.Sqrt)
            nc.vector.reciprocal(out=dinv[:], in_=dinv[:])

            # table-order degree tiles (consumer-side table builds)
            NA = NPAD // P
            sq_pm = pp.tile([P, NA], FP, tag="sqpm")
            dinv_pm = pp.tile([P, NA], FP, tag="dinvpm")
            nc.sync.dma_start(out=sq_pm[:], in_=degpm_d[:])
            nc.scalar.activation(out=sq_pm[:], in_=sq_pm[:],
                                 func=mybir.ActivationFunctionType.Sqrt)
            nc.vector.reciprocal(out=dinv_pm[:], in_=sq_pm[:])

            # ---------- big persistent buffers
            Z = pp.tile([F, SH], BF if zbf else FP, tag="z")  # pre-BN acts
            Z3 = pp.tile([P, J, n_cls], FP, tag="bigbuf")

            # ---------- DRAM internals (per-window AG pieces carry raw Z)
            wsz = meta["wsz"]
            NWIN = len(wsz)
            wend = []
            acc = 0
            for w in range(NWIN):
                acc += wsz[w]
                wend.append(acc // P - 1)          # last block of window w
            agins = [[dp.tile([wsz[w], F], BF, tag=f"agin{i}_{w}",
                              name=f"agin{i}_{w}")
                      for w in range(NWIN)] for i in range(1, 3)]
            zfulls = [None] + [
                [dp.tile([wsz[w] * N_CORES, F], BF, tag=f"zfull{i}_{w}",
                         name=f"zfull{i}_{w}", addr_space="Shared")
                 for w in range(NWIN)] for i in range(1, 3)]
            tables = [table0_d] + [
                [dp.tile([wsz[w] * N_CORES, F], BF, tag=f"table{i}_{w}",
                         name=f"table{i}_{w}")
                 for w in range(NWIN)] for i in range(1, 3)]
            st_in = dp.tile([P, 2], FP, tag="stin")
            st_outs = [dp.tile([P, 2], FP, tag=f"stout{i}",
                               name=f"stout{i}", addr_space="Shared")
                       for i in range(2)]
            scds = [dp.tile([F, 2], FP, tag=f"scd{i}", name=f"scd{i}")
                    for i in range(2)]

            if warm:
                # optional: absorb first-collective setup early (measured
                # neutral-to-negative; off by default)
                warm_in = dp.tile([P, 1], FP, tag="warmin")
                warm_out = dp.tile([P, 1], FP, tag="warmout", name="warmout",
                                   addr_space="Shared")
                wz = bp.tile([P, 1], FP, tag="wz")
                nc.vector.memset(wz[:], 0.0)
                nc.sync.dma_start(out=warm_in[:], in_=wz[:])
                nc.gpsimd.collective_compute(
                    "AllReduce", mybir.AluOpType.add,
                    replica_groups=[list(range(N_CORES))],
                    ins=[warm_in[:]], outs=[warm_out[:]])

            def table_ap(li, w, lo, hi):
                if li == 0:
                    return tables[0][lo:hi, :]
                return tables[li][w][:]

            # ---------- one GCN layer
            qctr = [0]

            def layer(w_sb, out_h, bias_col, z_dst, li):
                o16_base = 0
                col_base = 0
                for pl in plan:
                    slots = pl["slots"]
                    o16_len = sum(c[2] for c in pl["calls"])
                    idx_sb = gp.tile([P, O16_MAX], I16, tag="idxsb")
                    nc.sync.dma_start(
                        out=idx_sb[:, :o16_len],
                        in_=idx_d[:, o16_base:o16_base + o16_len])
                    dl_sb = gp.tile([P, SLOTS_MAX], BF, tag="dlsb")
                    nc.sync.dma_start(
                        out=dl_sb[:, :slots],
                        in_=dloc_d[:, col_base:col_base + slots])
                    strip = gp.tile([P, SLOTS_MAX, F], BF, tag="strip")
                    for (w, o16, n16, kstart) in pl["calls"]:
                        nidx = n16 * 16
                        lo = w * WIN
                        hi = min(NPAD, lo + WIN)
                        nc.gpsimd.dma_gather(
                            out_ap=strip[:, kstart:kstart + nidx // P, :],
                            in_ap=table_ap(li, w, lo, hi),
                            idxs_ap=idx_sb[:, o16:o16 + n16],
                            num_idxs=nidx, num_idxs_reg=nidx, elem_size=F,
                            queue_num=qctr[0] % 4)
                        qctr[0] += 1
                    for j in pl["blocks"]:
                        if ldepth < 1:
                            continue
                        chunks = pl["chunks"][j]
                        pagg = psp.tile([P, P], FP, tag="ps_agg")
                        nch = len(chunks)
                        # contiguous slot runs -> one batched one-hot build
                        runs = []
                        for t_in in chunks:
                            if runs and runs[-1][0] + runs[-1][1] == t_in:
                                runs[-1][1] += 1
                            else:
                                runs.append([t_in, 1])
                        i = 0
                        for (a, n) in runs:
                            S = sp.tile([P, RUNMAX, P], BF, tag="S")
                            nc.vector.tensor_tensor(
                                out=S[:, :n, :],
                                in0=dl_sb[:, a:a + n].to_broadcast([P, n, P]),
                                in1=iotar[:].rearrange(
                                    "p (s q) -> p s q", s=1
                                ).to_broadcast([P, n, P]),
                                op=mybir.AluOpType.is_equal)
                            for k in range(n):
                                nc.tensor.matmul(
                                    pagg[:], lhsT=S[:, k, :],
                                    rhs=strip[:, a + k, :],
                                    start=(i == 0), stop=(i == nch - 1))
                                i += 1
                        aggs = bp.tile([P, F], BF, tag="aggs")
                        nc.vector.tensor_scalar_mul(out=aggs[:], in0=pagg[:],
                                                    scalar1=dinv[:, j:j + 1])
                        if ldepth < 2:
                            continue
                        pt = psp.tile([P, P], BF, tag="ps_tb")
                        nc.tensor.transpose(out=pt[:], in_=aggs[:],
                                            identity=ident_bf[:])
                        aggT = bp.tile([P, F], BF, tag="aggT")
                        nc.vector.tensor_copy(out=aggT[:], in_=pt[:])
                        if ldepth < 3:
                            continue
                        pz = psp.tile([P, out_h], FP, tag="ps_z")
                        if out_h == n_cls:
                            nc.tensor.matmul(pz[:], lhsT=aggT[:], rhs=w_sb[:],
                                             start=True, stop=True)
                            nc.vector.tensor_add(out=z_dst[:, j, :], in0=pz[:],
                                                 in1=b3bc[:])
                        else:
                            nc.tensor.matmul(pz[:], lhsT=w_sb[:], rhs=aggT[:],
                                             start=True, stop=True)
                            nc.vector.tensor_scalar(
                                out=z_dst[:, j * P:(j + 1) * P], in0=pz[:],
                                scalar1=bias_col[:], scalar2=None,
                                op0=mybir.AluOpType.add)
                            if li < 2:
                                # ship raw Z block (node-major bf16) for the
                                # mid-layer AllGather of the next table
                                ptz = psp.tile([P, P], FP, tag="ps_tb")
                                nc.tensor.transpose(
                                    out=ptz[:],
                                    in_=z_dst[:, j * P:(j + 1) * P],
                                    identity=ident[:])
                                zb = bp.tile([P, F], BF, tag="zship")
                                nc.vector.tensor_copy(out=zb[:], in_=ptz[:])
                                s0 = j * P
                                w_, off = 0, s0
                                while off >= wsz[w_]:
                                    off -= wsz[w_]
                                    w_ += 1
                                nc.sync.dma_start(
                                    out=agins[li][w_][off:off + P, :],
                                    in_=zb[:])
                    if li < 2:
                        # window complete -> fire its AllGather piece now so
                        # it overlaps the rest of this layer's compute
                        for w in range(NWIN):
                            if wend[w] in pl["blocks"]:
                                nc.gpsimd.collective_compute(
                                    "AllGather", mybir.AluOpType.bypass,
                                    replica_groups=[list(range(N_CORES))],
                                    ins=[agins[li][w][:]],
                                    outs=[zfulls[li + 1][w][:]])
                    o16_base += o16_len
                    col_base += slots

            # ---------- BN coefficients (global stats -> scbc/shbc tiles)
            bnbc = [None, None]

            def bn_coeffs(g_col, be_col, li):
                st_out = st_outs[li]
                sub = math.gcd(512, SH)
                nsub = SH // sub
                stats = bp.tile([P, nsub, 6], FP, tag="bnst")
                zv = Z[:].rearrange("p (s q) -> p s q", s=nsub)
                for s in range(nsub):
                    nc.vector.bn_stats(out=stats[:, s, :], in_=zv[:, s, :])
                mv = bp.tile([P, 2], FP, tag="bnmv")
                nc.vector.bn_aggr(out=mv[:], in_=stats[:])
                sums = bp.tile([P, 2], FP, tag="sums")
                musq = bp.tile([P, 1], FP, tag="musq")
                nc.vector.tensor_mul(out=musq[:], in0=mv[:, 0:1], in1=mv[:, 0:1])
                nc.scalar.mul(out=sums[:, 0:1], in_=mv[:, 0:1], mul=float(SH))
                nc.vector.tensor_add(out=sums[:, 1:2], in0=mv[:, 1:2],
                                     in1=musq[:])
                nc.scalar.mul(out=sums[:, 1:2], in_=sums[:, 1:2], mul=float(SH))
                nc.sync.dma_start(out=st_in[:], in_=sums[:])
                nc.gpsimd.collective_compute(
                    "AllReduce", mybir.AluOpType.add,
                    replica_groups=[list(range(N_CORES))],
                    ins=[st_in[:]], outs=[st_out[:]])
                gl = bp.tile([P, 2], FP, tag="gl")
                nc.sync.dma_start(out=gl[:], in_=st_out[:])
                mu = bp.tile([P, 1], FP, tag="mu")
                var = bp.tile([P, 1], FP, tag="var")
                nc.scalar.mul(out=mu[:], in_=gl[:, 0:1], mul=1.0 / N)
                nc.scalar.mul(out=var[:], in_=gl[:, 1:2], mul=1.0 / N)
                nc.vector.tensor_mul(out=musq[:], in0=mu[:], in1=mu[:])
                nc.vector.tensor_sub(out=var[:], in0=var[:], in1=musq[:])
                rstd = bp.tile([P, 1], FP, tag="rstd")
                nc.scalar.activation(out=rstd[:], in_=var[:],
                                     func=mybir.ActivationFunctionType.Sqrt,
                                     bias=epsc[:], scale=1.0)
                nc.vector.reciprocal(out=rstd[:], in_=rstd[:])
                scsh = bp.tile([F, 2], FP, tag="scsh")
                nc.vector.tensor_mul(out=scsh[:, 0:1], in0=g_col[:],
                                     in1=rstd[:])
                nc.vector.tensor_mul(out=scsh[:, 1:2], in0=mu[:],
                                     in1=scsh[:, 0:1])
                nc.vector.tensor_sub(out=scsh[:, 1:2], in0=be_col[:],
                                     in1=scsh[:, 1:2])
                # roundtrip through DRAM to partition-broadcast sc/sh rows
                nc.sync.dma_start(out=scds[li][:], in_=scsh[:])
                base = scds[li][:]
                scbc = pp.tile([P, F], FP, tag=f"scbc{li}")
                shbc = pp.tile([P, F], FP, tag=f"shbc{li}")
                nc.sync.dma_start(
                    out=scbc[:],
                    in_=bass.AP(tensor=base.tensor, offset=base.offset,
                                ap=[[0, P], [2, F]]))
                nc.sync.dma_start(
                    out=shbc[:],
                    in_=bass.AP(tensor=base.tensor, offset=base.offset + 1,
                                ap=[[0, P], [2, F]]))
                bnbc[li] = (scbc, shbc)

            # ---------- consumer-side next-table build from AllGathered Z
            TBA = 8

            def table_build(cons):
                scbc, shbc = bnbc[cons - 1]
                scv = scbc[:].rearrange("p (s f) -> p s f", s=1)
                shv = shbc[:].rearrange("p (s f) -> p s f", s=1)
                for w in range(NWIN):
                    rows = wsz[w] * N_CORES
                    ntile = (rows + TBA * P - 1) // (TBA * P)
                    for it in range(ntile):
                        r0 = it * TBA * P
                        nr = min(TBA * P, rows - r0)
                        na = nr // P
                        a0 = (w * WIN + r0) // P
                        zt = bp.tile([P, TBA, F], BF, tag="tbz")
                        nc.sync.dma_start(
                            out=zt[:, :na, :],
                            in_=zfulls[cons][w][r0:r0 + nr, :].rearrange(
                                "(a p) f -> p a f", p=P))
                        tw = bp.tile([P, TBA, F], BF, tag="tbw")
                        nc.vector.tensor_tensor(
                            out=tw[:, :na, :], in0=zt[:, :na, :],
                            in1=scv.to_broadcast([P, na, F]),
                            op=mybir.AluOpType.mult)
                        nc.vector.tensor_tensor(
                            out=tw[:, :na, :], in0=tw[:, :na, :],
                            in1=shv.to_broadcast([P, na, F]),
                            op=mybir.AluOpType.add)
                        if cons == 1:
                            # residual: x recovered from table0 * sqrt(deg)
                            x0 = bp.tile([P, TBA, F], BF, tag="tbx")
                            t0 = w * WIN + r0
                            nc.sync.dma_start(
                                out=x0[:, :na, :],
                                in_=table0_d[t0:t0 + nr, :].rearrange(
                                    "(a p) f -> p a f", p=P))
                            xr = bp.tile([P, TBA, F], BF, tag="tbxr")
                            nc.vector.tensor_tensor(
                                out=xr[:, :na, :], in0=x0[:, :na, :],
                                in1=sq_pm[:, a0:a0 + na].to_broadcast(
                                    [P, na, F]),
                                op=mybir.AluOpType.mult)
                            nc.vector.tensor_tensor(
                                out=tw[:, :na, :], in0=tw[:, :na, :],
                                in1=xr[:, :na, :], op=mybir.AluOpType.add)
                        # dinv before relu (dinv > 0 commutes with relu)
                        nc.vector.tensor_tensor(
                            out=tw[:, :na, :], in0=tw[:, :na, :],
                            in1=dinv_pm[:, a0:a0 + na].to_broadcast(
                                [P, na, F]),
                            op=mybir.AluOpType.mult)
                        to = bp.tile([P, TBA, F], BF, tag="tbo")
                        nc.scalar.activation(
                            out=to[:, :na, :], in_=tw[:, :na, :],
                            func=mybir.ActivationFunctionType.Relu)
                        nc.sync.dma_start(
                            out=tables[cons][w][r0:r0 + nr, :].rearrange(
                                "(a p) f -> p a f", p=P),
                            in_=to[:, :na, :])

            # ================= layers
            def _early_out():
                nc.vector.memset(Z3[:], 0.0)
                nc.vector.tensor_add(out=Z3[:, 0, :1], in0=Z[:, 0:1],
                                     in1=Z[:, 1:2])
                nc.sync.dma_start(out=out_d[:], in_=Z3[:])

            if stage >= 2:
                nc.vector.memset(Z[:], 0.0)
                layer(w1, F, b1c, Z, 0)
            else:
                nc.vector.memset(Z[:], 0.0)
            if stage >= 3:
                bn_coeffs(g1c, be1c, 0)
            if stage >= 4:
                table_build(1)
            if stage >= 5:
                layer(w2, F, b2c, Z, 1)
                bn_coeffs(g2c, be2c, 1)
                table_build(2)
            if stage >= 6:
                layer(w3, n_cls, None, Z3, 2)

            if stage < 6:
                _early_out()
                do_softmax = False
            else:
                do_softmax = True
            # ---------- log_softmax over classes (free dim)
            zv = Z3[:]                                    # [P, J, C]
            if do_softmax:
                mx = bp.tile([P, J, 1], FP, tag="mx")
                nc.vector.reduce_max(out=mx[:], in_=zv,
                                     axis=mybir.AxisListType.X)
                nc.vector.tensor_sub(out=zv, in0=zv,
                                     in1=mx[:].to_broadcast([P, J, n_cls]))
                ex = pp.tile([P, J, n_cls], FP, tag="z")   # Z is dead here
                nc.scalar.activation(out=ex[:], in_=zv,
                                     func=mybir.ActivationFunctionType.Exp)
                sm = bp.tile([P, J, 1], FP, tag="sm")
                nc.vector.reduce_sum(out=sm[:], in_=ex[:],
                                     axis=mybir.AxisListType.X)
                ls = bp.tile([P, J, 1], FP, tag="ls")
                nc.scalar.activation(out=ls[:], in_=sm[:],
                                     func=mybir.ActivationFunctionType.Ln)
                nc.vector.tensor_sub(out=zv, in0=zv,
                                     in1=ls[:].to_broadcast([P, J, n_cls]))
                nc.sync.dma_start(out=out_d[:], in_=Z3[:])

    nc.compile()
    return nc


def _make_in_maps(meta, idx16, dloc, xs, degt, table0, xts, inputs):
    iota_rows = np.tile(np.arange(P, dtype=np.float32)[None, :], (P, 1))
    shared = dict(
        ident=np.eye(P, dtype=np.float32),
        iotar=iota_rows.astype(ml_dtypes.bfloat16),
        table0=table0,
        deg_pm=meta["deg_pm"],
        **{k: np.asarray(inputs[k], np.float32)
           for k in ("W1", "W2", "W3", "b1", "b2", "b3",
                     "g1", "be1", "g2", "be2")})
    return [dict(xtsh=xts[c], degt=degt[c], idx16=idx16[c], dloc=dloc[c],
                 **shared) for c in range(N_CORES)]


def _unshard(meta, results, n_cls):
    J, SH = meta["J"], meta["SH"]
    out = np.empty((meta["NPAD"], n_cls), np.float32)
    for c in range(N_CORES):
        o = results[c]["out"]                             # [P, J, C]
        nids = ((np.arange(J) * N_CORES + c)[:, None] * P
                + np.arange(P)[None, :])
        out[nids.reshape(-1)] = o.transpose(1, 0, 2).reshape(SH, n_cls)
    full = np.empty((meta["N"], n_cls), np.float32)
    full[meta["order"]] = out[:meta["N"]]
    return full


# ---------------------------------------------------------------- entry

def kernel(x, edge_index, W1, b1, g1, be1, W2, b2, g2, be2, W3, b3):
    x = np.asarray(x, dtype=np.float32)
    edge_index = np.asarray(edge_index)
    N, F = x.shape
    C = np.asarray(W3).shape[1]

    meta, idx16, dloc, xs, degt, table0, xts = _host_prep(x, edge_index, N)
    nc = _build(meta, F, C)
    in_maps = _make_in_maps(meta, idx16, dloc, xs, degt, table0, xts, dict(
        W1=W1, W2=W2, W3=W3, b1=b1, b2=b2, b3=b3,
        g1=g1, be1=be1, g2=g2, be2=be2))
    res = run_bass_kernel_spmd(nc, in_maps, core_ids=list(range(N_CORES)))
    return _unshard(meta, res.results, C)

